# revision 4
# baseline (speedup 1.0000x reference)
"""Trainium2 Bass kernel for nn_CharRNN: 2-layer MI-LSTM + large vocab projection.

Strategy (8 NeuronCores):
- Host: embedding gather, gate-block permutation [j|i|f|o], alpha absorbed into Uh,
  constants beta2' = beta2/alpha and gamma = bias' - beta1*beta2' so that the per-step
  gate math becomes g = (hh' + beta1) * A + gamma with A = xh + beta2' (3 vector ops).
- Device: batch-split LSTM (8 batch rows per core, gates kept transposed
  [G on partitions, batch on free]), per-segment AllGather of top-layer h,
  vocab-sharded output projection (6250 vocab columns per core) pipelined
  behind the LSTM segments.
- Host: concatenate the 8 [3200, 6250] shards along vocab, add softmax bias.
"""

import os
import sys

sys.path.insert(0, "/opt/trn_rl_repo")

import numpy as np

import concourse.bacc as bacc
import concourse.mybir as mybir
import concourse.tile as tile
from concourse import bass_utils

try:
    import jax

    jax.config.update("jax_compilation_cache_dir", "/root/jax_neff_cache")
    jax.config.update("jax_persistent_cache_min_entry_size_bytes", -1)
    jax.config.update("jax_persistent_cache_min_compile_time_secs", 1)
except Exception:
    pass


def _install_profile_hook():
    """Register the axon NTFF profile hook (the image's antenv stub lacks it)."""
    import types

    import antenv

    if "antenv.axon_hooks" not in sys.modules:
        mod = types.ModuleType("antenv.axon_hooks")
        mod._hook = None

        def set_axon_ntff_profile_hook(h):
            mod._hook = h

        def get_axon_ntff_profile_hook():
            return mod._hook

        mod.set_axon_ntff_profile_hook = set_axon_ntff_profile_hook
        mod.get_axon_ntff_profile_hook = get_axon_ntff_profile_hook
        sys.modules["antenv.axon_hooks"] = mod
        antenv.axon_hooks = mod
    from trn_agent_boot.trn_boot import _ntff_profile_via_ctypes

    sys.modules["antenv.axon_hooks"].set_axon_ntff_profile_hook(
        _ntff_profile_via_ctypes("/opt/axon/libaxon_pjrt.so")
    )
    # artifact upload has no bucket access in this container
    bass_utils.upload_artifacts = lambda tmpdir: f"local://{tmpdir}"

# problem constants (hardcoded per contract)
B, T, H, V = 64, 50, 256, 50000
G = 4 * H
N_CORES = 8
NB = B // N_CORES          # batch rows per core
VS = V // N_CORES          # vocab cols per core
SEG = 10                   # timesteps per segment
NSEG = T // SEG
ROWS = NB * T              # 400 local rows (t-major: t*NB + b)
GC = G // 128              # 8 gate chunks
HC = H // 128              # 2 hidden chunks
FORGET_BIAS = 1.0

F32 = mybir.dt.float32
AF = mybir.ActivationFunctionType

_COMPILED = {}
LAST_EXEC_NS = None


def _build():
    nc = bacc.Bacc(
        "TRN2",
        target_bir_lowering=False,
        debug=False,
        enable_asserts=True,
        num_devices=N_CORES,
    )

    # --- DRAM I/O (per-core data supplied via in_maps) ---
    d_xT = nc.dram_tensor("xT", [H, ROWS], F32, kind="ExternalInput")
    d_wx0 = nc.dram_tensor("wx0", [H, G], F32, kind="ExternalInput")
    d_wx1 = nc.dram_tensor("wx1", [H, G], F32, kind="ExternalInput")
    d_uh0 = nc.dram_tensor("uh0", [H, G], F32, kind="ExternalInput")
    d_uh1 = nc.dram_tensor("uh1", [H, G], F32, kind="ExternalInput")
    d_wt = nc.dram_tensor("wt", [H, VS], F32, kind="ExternalInput")
    d_b1g = nc.dram_tensor("b1g", [2, 128, GC, NB], F32, kind="ExternalInput")
    d_gg = nc.dram_tensor("gg", [2, 128, GC, NB], F32, kind="ExternalInput")
    d_b2s = nc.dram_tensor("b2s", [2, 128, GC], F32, kind="ExternalInput")
    d_out = nc.dram_tensor("out", [B * T, VS], F32, kind="ExternalOutput")

    with tile.TileContext(nc) as tc:
        with (
            tc.tile_pool(name="consts", bufs=1) as consts,
            tc.tile_pool(name="state", bufs=1) as state,
            tc.tile_pool(name="hseg", bufs=2) as hseg,
            tc.tile_pool(name="gates", bufs=4) as gates,
            tc.tile_pool(name="hg", bufs=2) as hgp,
            tc.tile_pool(name="ocopy", bufs=6) as ocopy,
            tc.tile_pool(name="pproj", bufs=3, space="PSUM") as pproj,
            tc.tile_pool(name="pgate", bufs=2, space="PSUM") as pgate,
            tc.tile_pool(name="dram", bufs=2, space="DRAM") as dram,
        ):
            # ---- load constants ----
            wt_sb = consts.tile([128, HC, VS], F32)
            nc.sync.dma_start(out=wt_sb, in_=d_wt.ap().rearrange("(k p) v -> p k v", p=128))
            uh_sb = []
            wx_sb = []
            for l, (duh, dwx) in enumerate(((d_uh0, d_wx0), (d_uh1, d_wx1))):
                u = consts.tile([128, HC, G], F32, tag=f"uh{l}", name=f"uh{l}")
                nc.sync.dma_start(out=u, in_=duh.ap().rearrange("(k p) g -> p k g", p=128))
                uh_sb.append(u)
                w = consts.tile([128, HC, G], F32, tag=f"wx{l}", name=f"wx{l}")
                nc.sync.dma_start(out=w, in_=dwx.ap().rearrange("(k p) g -> p k g", p=128))
                wx_sb.append(w)
            xT_sb = consts.tile([128, HC, ROWS], F32)
            nc.sync.dma_start(out=xT_sb, in_=d_xT.ap().rearrange("(k p) r -> p k r", p=128))
            b1g_sb = consts.tile([128, 2, GC, NB], F32)
            nc.sync.dma_start(out=b1g_sb, in_=d_b1g.ap().rearrange("l p m b -> p l m b"))
            gg_sb = consts.tile([128, 2, GC, NB], F32)
            nc.sync.dma_start(out=gg_sb, in_=d_gg.ap().rearrange("l p m b -> p l m b"))
            b2s_sb = consts.tile([128, 2, GC], F32)
            nc.sync.dma_start(out=b2s_sb, in_=d_b2s.ap().rearrange("l p m -> p l m"))

            # ---- persistent state ----
            zh = state.tile([128, HC, NB], F32)      # zero h for t=0
            nc.vector.memset(zh, 0.0)
            jc = []                                   # [tanh_j | c] per layer
            for l in range(2):
                t_jc = state.tile([128, 4, NB], F32, tag=f"jc{l}", name=f"jc{l}")
                nc.vector.memset(t_jc, 0.0)
                jc.append(t_jc)
            A_slab = []                               # A = xh + beta2', per layer
            for l in range(2):
                A_slab.append(state.tile([128, GC, ROWS], F32, tag=f"A{l}", name=f"A{l}"))

            # batch-major output view: row = b*T + t  ->  [t, b, v]
            out_r = d_out.ap().rearrange("(b t) v -> t b v", t=T)

            def batched_xh(l, rhs, rows, col0):
                """xh[l] for `rows` columns of rhs ([128, HC, rows]); writes A_slab[l][:, :, col0:col0+rows]."""
                for m in range(GC):
                    ps = pproj.tile([128, rows], F32, tag="ps", name="ps")
                    for k in range(HC):
                        nc.tensor.matmul(
                            ps,
                            wx_sb[l][:, k, m * 128:(m + 1) * 128],
                            rhs[:, k, :],
                            start=(k == 0),
                            stop=(k == HC - 1),
                        )
                    nc.vector.tensor_scalar(
                        out=A_slab[l][:, m, col0:col0 + rows],
                        in0=ps,
                        scalar1=b2s_sb[:, l, m:m + 1],
                        scalar2=None,
                        op0=mybir.AluOpType.add,
                    )

            def lstm_step(l, t, h_prev, h_out_slice):
                """One MI-LSTM step for layer l at global step t.
                h_prev: [128, HC, NB] AP; h_out_slice: destination AP [128, HC, NB]."""
                ph = pgate.tile([128, GC, NB], F32, tag="ph", name="ph")
                for m in range(GC):
                    for k in range(HC):
                        nc.tensor.matmul(
                            ph[:, m, :],
                            uh_sb[l][:, k, m * 128:(m + 1) * 128],
                            h_prev[:, k, :],
                            start=(k == 0),
                            stop=(k == HC - 1),
                        )
                v = gates.tile([128, GC, NB], F32, tag="v", name="v")
                nc.vector.tensor_add(v, ph, b1g_sb[:, l, :, :])
                u = gates.tile([128, GC, NB], F32, tag="u", name="u")
                nc.vector.tensor_mul(u, v, A_slab[l][:, :, t * NB:(t + 1) * NB])
                g = gates.tile([128, GC, NB], F32, tag="g", name="g")
                nc.vector.tensor_add(g, u, gg_sb[:, l, :, :])
                # gate order [j j i i f f o o] by 128-chunks
                nc.scalar.activation(jc[l][:, 0:2, :], g[:, 0:2, :], AF.Tanh)
                s_t = gates.tile([128, 6, NB], F32, tag="s", name="s_t")
                nc.scalar.activation(s_t, g[:, 2:8, :], AF.Sigmoid)
                prod = gates.tile([128, 4, NB], F32, tag="prod", name="prod")
                nc.vector.tensor_mul(prod, s_t[:, 0:4, :], jc[l])
                nc.vector.tensor_add(jc[l][:, 2:4, :], prod[:, 0:2, :], prod[:, 2:4, :])
                th = gates.tile([128, 2, NB], F32, tag="th", name="th")
                nc.scalar.activation(th, jc[l][:, 2:4, :], AF.Tanh)
                nc.vector.tensor_mul(h_out_slice, th, s_t[:, 4:6, :])

            # ---- initial xh0 for ALL steps ----
            batched_xh(0, xT_sb, ROWS, 0)

            prev_h = [zh[:], zh[:]]
            h1_tiles = []
            for s in range(NSEG):
                # layer 0, segment s
                h0s = hseg.tile([128, HC, SEG * NB], F32, tag="h0seg", name="h0s")
                for tin in range(SEG):
                    t = s * SEG + tin
                    sl = h0s[:, :, tin * NB:(tin + 1) * NB]
                    lstm_step(0, t, prev_h[0], sl)
                    prev_h[0] = sl
                # xh1 for segment s
                batched_xh(1, h0s.rearrange("p c r -> p c r"), SEG * NB, s * SEG * NB)
                # layer 1, segment s
                h1s = hseg.tile([128, HC, SEG * NB], F32, tag="h1seg", name="h1s")
                for tin in range(SEG):
                    t = s * SEG + tin
                    sl = h1s[:, :, tin * NB:(tin + 1) * NB]
                    lstm_step(1, t, prev_h[1], sl)
                    prev_h[1] = sl
                h1_tiles.append(h1s)

                # AllGather this segment's h1 across cores
                agin = dram.tile([128, HC, SEG * NB], F32, tag="agin", name="agin")
                agout = dram.tile([N_CORES, 128, HC, SEG, NB], F32, tag="agout", name="agout")
                nc.sync.dma_start(out=agin, in_=h1s)
                nc.gpsimd.collective_compute(
                    "AllGather",
                    mybir.AluOpType.bypass,
                    replica_groups=[list(range(N_CORES))],
                    ins=[agin.opt()],
                    outs=[agout.opt()],
                )
                hg = hgp.tile([128, HC, SEG, N_CORES, NB], F32, tag="hg", name="hg")
                nc.sync.dma_start(
                    out=hg, in_=agout[:].rearrange("r p c t b -> p c t r b")
                )

                # projection for segment s: rows (t, b_global) for t in seg
                for mt in range(SEG // 2):
                    t0 = 2 * mt
                    lhs = [hg[:, k, t0:t0 + 2, :, :] for k in range(HC)]
                    for pair in range(7):
                        if pair < 6:
                            n0, wtot = pair * 1024, 1024
                        else:
                            n0, wtot = 6144, VS - 6144
                        ps = pproj.tile([128, wtot], F32, tag="ps", name="ps")
                        nhalf = 2 if pair < 6 else 1
                        for half in range(nhalf):
                            nn = n0 + half * 512
                            ww = min(512, wtot - half * 512)
                            for k in range(HC):
                                nc.tensor.matmul(
                                    ps[:, half * 512:half * 512 + ww],
                                    lhs[k],
                                    wt_sb[:, k, nn:nn + ww],
                                    start=(k == 0),
                                    stop=(k == HC - 1),
                                )
                        ob = ocopy.tile([128, wtot], F32, tag="ob", name="ob")
                        if (mt * 7 + pair) % 2 == 0:
                            nc.vector.tensor_copy(ob, ps)
                        else:
                            nc.scalar.copy(ob, ps)
                        nc.sync.dma_start(
                            out=out_r[s * SEG + t0: s * SEG + t0 + 2, :, n0:n0 + wtot],
                            in_=ob,
                        )

    nc.compile()
    return nc


def _prep_inputs(inputs):
    """Host-side preprocessing -> per-core in_maps."""
    idx = np.asarray(inputs["input_data"]).astype(np.int64)
    emb = np.asarray(inputs["embedding"], dtype=np.float32)
    Wx = np.asarray(inputs["Wx"], dtype=np.float32)
    Uh = np.asarray(inputs["Uh"], dtype=np.float32)
    alpha = np.asarray(inputs["alpha"], dtype=np.float32)
    beta1 = np.asarray(inputs["beta1"], dtype=np.float32)
    beta2 = np.asarray(inputs["beta2"], dtype=np.float32)
    bias = np.asarray(inputs["bias"], dtype=np.float32)
    sw = np.asarray(inputs["softmax_w"], dtype=np.float32)

    perm = np.concatenate(
        [np.arange(256, 512), np.arange(0, 256), np.arange(512, 768), np.arange(768, 1024)]
    )
    Wx_p = np.ascontiguousarray(Wx[:, :, perm])
    Uh_p = Uh[:, :, perm]
    alpha_p = alpha[:, perm]
    beta1_p = beta1[:, perm]
    beta2_p = beta2[:, perm]
    bias_p = bias[:, perm].copy()
    bias_p[:, 512:768] += FORGET_BIAS

    Uh_abs = np.ascontiguousarray(Uh_p * alpha_p[:, None, :])
    beta2p = beta2_p / alpha_p
    gamma = bias_p - beta1_p * beta2p

    # [2, 128, GC, NB] per-(p, chunk) constants replicated along local batch
    def pcn(arr):  # [2, G] -> [2, 128, GC]
        return np.ascontiguousarray(arr.reshape(2, GC, 128).transpose(0, 2, 1))

    b1g = np.ascontiguousarray(
        np.repeat(pcn(beta1_p)[:, :, :, None], NB, axis=3)
    ).astype(np.float32)
    gg = np.ascontiguousarray(
        np.repeat(pcn(gamma)[:, :, :, None], NB, axis=3)
    ).astype(np.float32)
    b2s = pcn(beta2p).astype(np.float32)

    x = emb[idx]                      # [B, T, H]
    in_maps = []
    for c in range(N_CORES):
        xc = x[c * NB:(c + 1) * NB]   # [NB, T, H]
        xT = np.ascontiguousarray(xc.transpose(2, 1, 0).reshape(H, ROWS))
        wtc = np.ascontiguousarray(sw[c * VS:(c + 1) * VS, :].T)  # [H, VS]
        in_maps.append(
            {
                "xT": xT,
                "wx0": Wx_p[0],
                "wx1": Wx_p[1],
                "uh0": Uh_abs[0],
                "uh1": Uh_abs[1],
                "wt": wtc,
                "b1g": b1g,
                "gg": gg,
                "b2s": b2s,
            }
        )
    return in_maps


def kernel(**inputs) -> np.ndarray:
    global LAST_EXEC_NS
    if "nc" not in _COMPILED:
        _COMPILED["nc"] = _build()
    nc = _COMPILED["nc"]
    in_maps = _prep_inputs(inputs)
    trace = bool(int(os.environ.get("KERNEL_TRACE", "0")))
    kw = {}
    if trace:
        try:
            _install_profile_hook()
            kw["tmpdir"] = os.environ.get("KERNEL_TRACE_DIR") or None
        except Exception as e:
            print(f"profile hook unavailable ({e}); running untraced", file=sys.stderr)
            trace = False
    res = bass_utils.run_bass_kernel_spmd(
        nc, in_maps, core_ids=list(range(N_CORES)), trace=trace, **kw
    )
    LAST_EXEC_NS = res.exec_time_ns
    sb = np.asarray(inputs["softmax_b"], dtype=np.float32)
    out = np.empty((B * T, V), np.float32)
    for c in range(N_CORES):
        out[:, c * VS:(c + 1) * VS] = res.results[c]["out"]
    out += sb[None, :]
    return out
